# revision 45
# baseline (speedup 1.0000x reference)
"""Bass/Trainium2 kernel for a 3-layer GCN (GCNConv x2 + Linear).

Contract: kernel(**inputs) takes the FULL unsharded inputs
(x [N,128] f32, edge_index [2,E] i64, W1,b1,W2,b2,Wf,bf) and returns the
FULL [N,64] f32 output, distributing work across 8 NeuronCores internally.

Math: PyG GCNConv with self loops,
    gcn(x) = Dinv (A + I) Dinv (x W) + b,   Dinv = diag(1/sqrt(deg))
Aggregation and the dense transform commute, so each layer is computed as
    z = dinv * (A @ (dinv * h) + dinv * h);  h' = relu(z @ W + b)
The device gathers rows of a pre-scaled fp16 table (dma_gather on 4 SWDGE
queues, int16 bank-local indices), scatter-adds them into PSUM via one-hot
matmuls on the TensorEngine (one wide broadcast-AP is_equal per
(group, bank) builds all one-hots), applies the self term + dinv scale on
DVE, and runs the small dense matmul locally per 512-node group.

Sharding: destination nodes are sharded 8 ways with a degree-stratified
permutation (nodes dealt round-robin by in-degree to cores, then to tiles)
so per-(slot, bank) segment capacities are near-uniform across the 8 SPMD
cores; segments are 16-index granular with idx-0/dst-300 padding.  Each
core uploads only its own x shard; bank stripes of the table are built
on-device by per-bank AllGathers (and likewise for the layer-2 table,
letting stripes fire as layer-1 groups complete).  The final output is
written feature-major and un-permuted on the host.
"""

import os

import numpy as np

P = 128
N_CORES = 8
GW = 512         # dense-matmul group width = 4 dst tiles (one PSUM bank)
# dma_gather idx is int16: bank the table. 20000 -> 5 banks, which keeps
# each (group, bank) index span under 2048 = two full single_packet calls.
BANK_MAX = int(os.environ.get("GCN_BANK_MAX", "32000"))
CAP_CHUNKS = int(os.environ.get("GCN_CAP_CHUNKS", "8"))  # max chunks/gather
# single_packet packs each SDMA engine's descriptors into one packet (fast
# Q7 generation) but is limited to 64 descs/engine = 1024 indices/gather.
SINGLE_PACKET = os.environ.get("GCN_SINGLE_PACKET", "1") == "1"
N_SWDGE_QUEUES = int(os.environ.get("GCN_SWDGE_QUEUES", "4"))
DMA_SCRATCH = int(os.environ.get("GCN_DMA_SCRATCH", "16384"))

_LAST = {}  # diagnostics from the most recent kernel() call


# ----------------------------------------------------------------- host prep
def _preprocess(x, edge_index, n_cores=N_CORES, bank_max=BANK_MAX):
    N, F = x.shape
    assert N % n_cores == 0
    shard = N // n_cores
    n_tiles = (shard + P - 1) // P
    last_nk = shard - (n_tiles - 1) * P
    n_groups = (shard + GW - 1) // GW
    n_banks = max(1, -(-N // bank_max))
    bank_size = -(-N // n_banks)

    qrows = shard // n_banks  # rows per core per bank stripe
    assert qrows * n_banks == shard

    src = np.asarray(edge_index[0], dtype=np.int64)
    dst = np.asarray(edge_index[1], dtype=np.int64)

    deg = np.bincount(dst, minlength=N).astype(np.float32) + 1.0
    dinv = (1.0 / np.sqrt(deg)).astype(np.float32)

    # stratified permutation: nodes sorted by in-degree desc, dealt
    # round-robin to cores (cross-core balance), then within each core dealt
    # round-robin to tiles (per-tile balance) so every (core, tile) has a
    # near-identical degree mix -> uniform segment capacities.
    deg_order = np.argsort(-deg, kind="stable")
    nk_of_slot = np.full(n_tiles, P, dtype=np.int64)
    nk_of_slot[n_tiles - 1] = last_nk
    pairs = [
        (pos, slot)
        for pos in range(P)
        for slot in range(n_tiles)
        if pos < nk_of_slot[slot]
    ]
    assert len(pairs) == shard
    perm = np.zeros(N, dtype=np.int64)
    for j, (pos, slot) in enumerate(pairs):
        for c in range(n_cores):
            perm[c * shard + slot * P + pos] = deg_order[n_cores * j + c]
    perm_inv = np.zeros(N, dtype=np.int64)
    perm_inv[perm] = np.arange(N)

    new_src = perm_inv[src]
    new_dst = perm_inv[dst]
    core_of = new_dst // shard
    slot_of = (new_dst % shard) // P
    dloc_of = (new_dst % shard) % P
    # bank b = stripe {rows c*shard + b*qrows + [0, qrows) for all c}, laid
    # out rank-major [c*qrows + r] — exactly one AllGather's output. Bank
    # tables have n_cores*qrows rows, which must stay int16-addressable.
    assert n_cores * qrows <= 32767
    e_bank = (new_src % shard) // qrows
    e_bidx = (new_src // shard) * qrows + (new_src % shard) % qrows

    # per-(core, slot, bank) segment counts
    seg = np.zeros((n_cores, n_tiles, n_banks), dtype=np.int64)
    for c in range(n_cores):
        m = core_of == c
        sb = slot_of[m] * n_banks + e_bank[m]
        seg[c] = np.bincount(sb, minlength=n_tiles * n_banks).reshape(
            n_tiles, n_banks
        )
    assert seg.sum(axis=(1, 2)).min() > 0

    # capacity per (slot, bank): max over cores, rounded up to 16 indices
    cap16 = ((seg.max(axis=0) + 15) // 16) * 16  # [n_tiles, n_banks]

    # idx-space layout: for g: for b: slots of g packed back-to-back
    off16 = np.zeros((n_tiles, n_banks), dtype=np.int64)  # local in (g,b)
    gb_q0 = np.zeros((n_groups, n_banks), dtype=np.int64)  # global idx start
    gb_span16 = np.zeros((n_groups, n_banks), dtype=np.int64)
    gb_nblk = np.zeros((n_groups, n_banks), dtype=np.int64)
    qtot = 0
    for g in range(n_groups):
        k_lo, k_hi = 4 * g, min(4 * g + 4, n_tiles)
        for b in range(n_banks):
            # pad the span to a 128 multiple (via the last slot's capacity)
            # so every payload block is fully gathered — ungathered SBUF can
            # be NaN, and NaN * 0 would poison the masked matmul.
            span = int(cap16[k_lo:k_hi, b].sum())
            cap16[k_hi - 1, b] += (-span) % P
            gb_q0[g, b] = qtot
            o = 0
            for k in range(k_lo, k_hi):
                off16[k, b] = o
                o += cap16[k, b]
            gb_span16[g, b] = o
            gb_nblk[g, b] = o // P
            qtot += o

    # matmul pairs: (g, b, slot, 128-block) -> S column m; contiguous per
    # (g, b) so one wide is_equal generates all of a (g, b)'s one-hots.
    gb_m0 = np.zeros((n_groups, n_banks), dtype=np.int64)
    gb_mcnt = np.zeros((n_groups, n_banks), dtype=np.int64)
    pair_of = {}
    mm_seq = {}  # (g, k) -> list of (b, payload block, S block)
    M = 0
    for g in range(n_groups):
        k_lo, k_hi = 4 * g, min(4 * g + 4, n_tiles)
        for b in range(n_banks):
            gb_m0[g, b] = M
            for k in range(k_lo, k_hi):
                if cap16[k, b] == 0:
                    continue
                blo = off16[k, b] // P
                bhi = (off16[k, b] + cap16[k, b] - 1) // P
                for ch in range(blo, bhi + 1):
                    pair_of[(g, b, k, ch)] = M
                    M += 1
            gb_mcnt[g, b] = M - gb_m0[g, b]
        for k in range(k_lo, k_hi):
            sq = []
            for b in range(n_banks):
                if cap16[k, b] == 0:
                    continue
                blo = off16[k, b] // P
                bhi = (off16[k, b] + cap16[k, b] - 1) // P
                for ch in range(blo, bhi + 1):
                    sq.append((b, ch, pair_of[(g, b, k, ch)] - gb_m0[g, b]))
            mm_seq[(g, k)] = sq

    # fill idx and dst_loc per core (pad positions: idx 0, dst_loc 300)
    n_qcols = qtot // 16
    idx_w = np.zeros((n_cores, 16, n_qcols), dtype=np.int16)
    dst_loc = np.full((n_cores, P, M), 300.0, dtype=np.float16)
    g_of_slot = np.arange(n_tiles) // 4
    for c in range(n_cores):
        m = np.where(core_of == c)[0]
        o = m[np.lexsort((e_bank[m], slot_of[m]))]
        ks, bs, rows, dl = slot_of[o], e_bank[o], e_bidx[o], dloc_of[o]
        sb = ks * n_banks + bs
        seg_sizes = np.bincount(sb, minlength=n_tiles * n_banks)
        seg_off = np.concatenate([[0], np.cumsum(seg_sizes)])
        j = np.arange(len(o)) - seg_off[sb]  # rank within segment
        gs_ = g_of_slot[ks]
        ql = off16[ks, bs] + j  # local idx position within (g, b)
        q = gb_q0[gs_, bs] + ql  # global idx position
        idx_w[c, q % 16, q // 16] = rows.astype(np.int16)
        mcols = np.array(
            [pair_of[(gs_[i], bs[i], ks[i], ql[i] // P)]
             for i in range(len(o))],
            dtype=np.int64,
        )
        dst_loc[c, ql % P, mcols] = dl

    return dict(
        N=N, F=F, E=len(src), n_cores=n_cores, shard=shard, n_tiles=n_tiles,
        last_nk=last_nk, nk_of_slot=nk_of_slot, n_groups=n_groups,
        n_banks=n_banks, bank_size=bank_size, qrows=qrows,
        cap16=cap16, off16=off16, gb_q0=gb_q0, gb_span16=gb_span16,
        gb_nblk=gb_nblk, gb_m0=gb_m0, gb_mcnt=gb_mcnt, mm_seq=mm_seq,
        n_qcols=n_qcols, M=M, qtot=qtot,
        perm=perm, perm_inv=perm_inv, dst_loc=dst_loc, idx_w=idx_w,
        dinv=dinv, pad_overhead=qtot * n_cores / max(len(src), 1),
    )


# ------------------------------------------------------------ device program
def _build(meta, fout, debug=False, enable_asserts=False, dbg_outs=False):
    from concourse import bacc, bass, mybir, tile

    dt = mybir.dt
    f16, f32, i16 = dt.float16, dt.float32, dt.int16
    Alu = mybir.AluOpType
    Act = mybir.ActivationFunctionType

    N, F = meta["N"], meta["F"]
    shard, n_tiles = meta["shard"], meta["n_tiles"]
    nk_of_slot = meta["nk_of_slot"]
    n_groups, n_banks = meta["n_groups"], meta["n_banks"]
    bank_size = meta["bank_size"]
    gb_q0, gb_span16 = meta["gb_q0"], meta["gb_span16"]
    gb_nblk = meta["gb_nblk"]
    gb_m0, gb_mcnt = meta["gb_m0"], meta["gb_mcnt"]
    mm_seq = meta["mm_seq"]
    n_qcols, M = meta["n_qcols"], meta["M"]
    n_cores = meta["n_cores"]
    max_nblk = int(gb_nblk.max())
    max_mcnt = int(gb_mcnt.max())

    nc = bacc.Bacc(
        "TRN2",
        target_bir_lowering=False,
        debug=debug,
        enable_asserts=enable_asserts,
        num_devices=n_cores,
        num_swdge_queues=N_SWDGE_QUEUES,
        dynamic_dma_scratch_size=DMA_SCRATCH,
    )

    x_shard_in = nc.dram_tensor("x_shard", [shard, F], f16, kind="ExternalInput")
    idx16 = nc.dram_tensor("idx16", [16, n_qcols], i16, kind="ExternalInput")
    dst_loc = nc.dram_tensor("dst_loc", [P, M], f16, kind="ExternalInput")
    dinv16 = nc.dram_tensor("dinv16", [16, shard], f16, kind="ExternalInput")
    j_const = nc.dram_tensor("j_const", [P, P], f16, kind="ExternalInput")
    ident_in = nc.dram_tensor("ident_in", [P, P], f16, kind="ExternalInput")
    w1 = nc.dram_tensor("w1", [F, F], f16, kind="ExternalInput")
    w2 = nc.dram_tensor("w2", [F, F], f16, kind="ExternalInput")
    wf = nc.dram_tensor("wf", [F, fout], f16, kind="ExternalInput")
    b1 = nc.dram_tensor("b1", [F, 1], f32, kind="ExternalInput")
    b2 = nc.dram_tensor("b2", [F, 1], f32, kind="ExternalInput")
    bf = nc.dram_tensor("bf", [fout, 1], f32, kind="ExternalInput")
    outT = nc.dram_tensor("outT", [fout, shard], f32, kind="ExternalOutput")

    qrows = meta["qrows"]
    bank_rows = n_cores * qrows
    shard_dram = nc.dram_tensor("shard_dram", [shard, F], f16)
    x_stage = nc.dram_tensor("x_stage", [shard, F], f16)
    x_q = [
        nc.dram_tensor(f"x_q{q}", [bank_rows, F], f16, addr_space="Shared")
        for q in range(n_banks)
    ]
    s2_q = [
        nc.dram_tensor(f"s2_q{q}", [bank_rows, F], f16, addr_space="Shared")
        for q in range(n_banks)
    ]

    def bank_ap(table, b):
        return table[b][:, :]

    with tile.TileContext(nc) as tc:
        with (
            tc.tile_pool(name="res", bufs=1) as res,
            tc.tile_pool(name="gat", bufs=2 * n_banks) as gat,
            tc.tile_pool(name="sgen", bufs=n_banks + 2) as sgen,
            tc.tile_pool(name="stg", bufs=4) as stg,
            tc.tile_pool(name="zp", bufs=3) as zp,
            tc.tile_pool(name="h2p", bufs=3) as h2p,
            tc.tile_pool(name="xgp", bufs=8) as xgp,
            tc.tile_pool(name="ps_agg", bufs=3, space="PSUM") as ps_agg,
            tc.tile_pool(name="ps_mm", bufs=2, space="PSUM") as ps_mm,
            tc.tile_pool(name="ps_tp", bufs=2, space="PSUM") as ps_tp,
        ):
            # ---- residents
            dl_sb = res.tile([P, M], f16, name="dl_sb")
            j_sb = res.tile([P, P], f16, name="j_sb")
            ident = res.tile([P, P], f16, name="ident")
            dinv_sb = res.tile([P, shard], f16, name="dinv_sb")
            sT1 = res.tile([P, shard], f16, name="sT1")
            sT2 = res.tile([P, shard], f16, name="sT2")
            ix_res = res.tile([P, n_qcols], i16, name="ix_res")
            w1_sb = res.tile([F, F], f16, name="w1_sb")
            w2_sb = res.tile([F, F], f16, name="w2_sb")
            wf_sb = res.tile([F, fout], f16, name="wf_sb")
            b1_sb = res.tile([F, 1], f32, name="b1_sb")
            b2_sb = res.tile([F, 1], f32, name="b2_sb")
            bf_sb = res.tile([fout, 1], f32, name="bf_sb")
            # replicate the table from per-core shards, one AllGather per
            # bank stripe, first thing (collectives cannot read IO tensors
            # directly -> stage through internal dram). Everything below
            # overlaps with the collectives.
            nc.sync.dma_start(out=x_stage[:, :], in_=x_shard_in[:, :])
            for q in range(n_banks):
                nc.gpsimd.collective_compute(
                    "AllGather",
                    mybir.AluOpType.bypass,
                    replica_groups=[list(range(n_cores))],
                    ins=[x_stage[q * qrows : (q + 1) * qrows, :]],
                    outs=[x_q[q].ap().opt()],
                )
            # indices + dinv: replicate the 16-row band to all 128 partitions
            for r in range(8):
                nc.sync.dma_start(
                    out=ix_res[16 * r : 16 * (r + 1), :], in_=idx16[:, :]
                )
                nc.sync.dma_start(
                    out=dinv_sb[16 * r : 16 * (r + 1), :], in_=dinv16[:, :]
                )
            for sb, dr in [
                (dl_sb, dst_loc), (j_sb, j_const), (ident, ident_in),
                (w1_sb, w1), (w2_sb, w2), (wf_sb, wf),
                (b1_sb, b1), (b2_sb, b2), (bf_sb, bf),
            ]:
                nc.sync.dma_start(out=sb[:], in_=dr[:, :])

            # self-term for layer 1: transpose the local shard on-chip.
            # Loads go through the scalar queue to keep Sync free.
            for k in range(n_tiles):
                nk = int(nk_of_slot[k])
                lo = k * P
                xt = xgp.tile([P, P], f16, name="xt", tag="xg")
                nc.scalar.dma_start(out=xt[:nk, :],
                                    in_=x_shard_in[lo : lo + nk, :])
                tp0 = ps_tp.tile([P, P], f16, name="tp0", tag="tp")
                nc.tensor.transpose(
                    out=tp0[:, :nk], in_=xt[:nk, :], identity=ident[:nk, :nk]
                )
                nc.scalar.activation(
                    out=sT1[:, lo : lo + nk], in_=tp0[:, :nk], func=Act.Copy
                )

            dbg = os.environ.get("GCN_DBG_MODE", "")
            qctr = [0]

            def emit_layer(layer, table, w_sb, b_sb):
                for g in range(n_groups):
                    gs = g * GW
                    ge = min(gs + GW, shard)
                    gw = ge - gs
                    k_lo, k_hi = 4 * g, min(4 * g + 4, n_tiles)
                    # gathers for this group, one tile per bank
                    gts = {}
                    for b in range(n_banks):
                        span16 = int(gb_span16[g, b])
                        if span16 == 0:
                            continue
                        q0 = int(gb_q0[g, b])
                        gt = gat.tile([P, max_nblk * F], f16,
                                      name="gt", tag="gt")
                        s = 0
                        while s < span16:
                            n = min(CAP_CHUNKS * P, span16 - s)
                            nblk = -(-n // P)
                            nc.gpsimd.dma_gather(
                                gt[:, (s // P) * F : (s // P + nblk) * F]
                                .rearrange("p (c f) -> p c f", f=F),
                                bank_ap(table, b),
                                ix_res[:, (q0 + s) // 16 :
                                       (q0 + s + n) // 16],
                                n, n, F,
                                single_packet=SINGLE_PACKET,
                                queue_num=qctr[0] % N_SWDGE_QUEUES,
                            )
                            qctr[0] += 1
                            s += n
                        gts[b] = gt
                    if dbg == "gonly":
                        continue
                    # one-hot S tiles for a whole (group, bank) in a single
                    # wide DVE op: S[p, m, j] = (j == dl[p, m]).
                    sgs = {}
                    for b in range(n_banks):
                        mcnt = int(gb_mcnt[g, b])
                        if mcnt == 0:
                            continue
                        m0 = int(gb_m0[g, b])
                        st = sgen.tile([P, max_mcnt * P], f16,
                                       name="st", tag="S")
                        if dbg != "nosgen":
                            out_ap = st[:, : mcnt * P].rearrange(
                                "p (c j) -> p c j", j=P
                            )
                            in0 = j_sb[:, :].unsqueeze(1).broadcast_to(
                                [P, mcnt, P]
                            )
                            in1 = dl_sb[:, m0 : m0 + mcnt].unsqueeze(
                                2
                            ).broadcast_to([P, mcnt, P])
                            nc.vector.tensor_tensor(
                                out=out_ap, in0=in0, in1=in1,
                                op=Alu.is_equal,
                            )
                        sgs[b] = st
                    # self-term source
                    own = sT1 if layer == 1 else sT2
                    own_lo = gs
                    zg = zp.tile([P, GW], f16, name="zg", tag="zg")
                    ps = ps_agg.tile([P, GW], f32, name="ps", tag="agg")
                    for k in range(k_lo, k_hi):
                        nk = int(nk_of_slot[k])
                        lo = k * P
                        kk = lo - gs  # column offset within the group
                        seq = mm_seq[(g, k)]
                        for i, (b, pos, mrel) in enumerate(seq):
                            nc.tensor.matmul(
                                out=ps[:, kk : kk + P],
                                lhsT=gts[b][:, pos * F : (pos + 1) * F],
                                rhs=sgs[b][:, mrel * P : (mrel + 1) * P],
                                start=(i == 0),
                                stop=(i == len(seq) - 1),
                            )
                    # z = (agg + self) * dinv for the whole group at once
                    ztmp = stg.tile([P, GW], f32, name="ztmp", tag="ztmp")
                    nc.vector.tensor_tensor(
                        out=ztmp[:, :gw],
                        in0=ps[:, :gw],
                        in1=own[:, own_lo : own_lo + gw],
                        op=Alu.add,
                    )
                    nc.vector.tensor_tensor(
                        out=zg[:, :gw],
                        in0=ztmp[:, :gw],
                        in1=dinv_sb[:, gs:ge],
                        op=Alu.mult,
                    )
                    # dense transform for the group
                    hp = ps_mm.tile([P, GW], f32, name="hp", tag="mm")
                    nc.tensor.matmul(
                        out=hp[:, :gw], lhsT=w_sb[:], rhs=zg[:, :gw],
                        start=True, stop=True,
                    )
                    if layer == 1:
                        hs = stg.tile([P, GW], f16, name="hs", tag="hs")
                        nc.scalar.activation(
                            out=hs[:, :gw], in_=hp[:, :gw], func=Act.Relu,
                            bias=b_sb[:, :1],
                        )
                        nc.vector.tensor_tensor(
                            out=sT2[:, gs:ge], in0=hs[:, :gw],
                            in1=dinv_sb[:, gs:ge], op=Alu.mult,
                        )
                        for k in range(k_lo, k_hi):
                            nk = int(nk_of_slot[k])
                            lo = k * P
                            tp = ps_tp.tile([P, P], f16, name="tp", tag="tp")
                            nc.tensor.transpose(
                                out=tp[:nk, :],
                                in_=sT2[:, lo : lo + nk],
                                identity=ident[:],
                            )
                            ts = stg.tile([P, P], f16, name="ts", tag="ts")
                            nc.scalar.activation(
                                out=ts[:nk, :], in_=tp[:nk, :],
                                func=Act.Copy,
                            )
                            nc.sync.dma_start(
                                out=shard_dram[lo : lo + nk, :],
                                in_=ts[:nk, :],
                            )
                    else:
                        h2g = h2p.tile([P, GW], f16, name="h2g", tag="h2")
                        nc.scalar.activation(
                            out=h2g[:, :gw], in_=hp[:, :gw], func=Act.Relu,
                            bias=b_sb[:, :1],
                        )
                        op = ps_mm.tile([fout, GW], f32, name="op", tag="mm")
                        nc.tensor.matmul(
                            out=op[:, :gw], lhsT=wf_sb[:], rhs=h2g[:, :gw],
                            start=True, stop=True,
                        )
                        os_ = stg.tile([fout, GW], f32, name="os_", tag="os")
                        nc.scalar.activation(
                            out=os_[:, :gw], in_=op[:, :gw],
                            func=Act.Identity, bias=bf_sb[:, :1],
                        )
                        nc.sync.dma_start(out=outT[:, gs:ge],
                                          in_=os_[:, :gw])

            dbg_mode = os.environ.get("GCN_DBG_MODE", "")
            reps = int(os.environ.get("GCN_REPEAT", "1"))
            for _rep in range(reps):
                emit_layer(1, x_q, w1_sb, b1_sb)
                if dbg_mode != "noag":
                    # per-bank AllGather: each stripe fires as soon as its
                    # shard_dram rows are written, overlapping layer-1
                    # compute; only the last stripe gates layer 2's tail.
                    for q in range(n_banks):
                        nc.gpsimd.collective_compute(
                            "AllGather",
                            mybir.AluOpType.bypass,
                            replica_groups=[list(range(n_cores))],
                            ins=[
                                shard_dram[q * qrows : (q + 1) * qrows, :]
                            ],
                            outs=[s2_q[q].ap().opt()],
                        )
                l2_tab = x_q if dbg_mode in ("noag", "l2x") else s2_q
                emit_layer(2, l2_tab, w2_sb, b2_sb)

            if dbg_outs:
                d_sT2 = nc.dram_tensor("d_sT2", [P, shard], f16,
                                       kind="ExternalOutput")
                nc.sync.dma_start(out=d_sT2[:, :], in_=sT2[:])

    nc.compile()
    return nc


def _make_in_maps(meta, x, W1, b1, W2, b2, Wf, bf):
    shard, n_cores = meta["shard"], meta["n_cores"]
    perm, dinv = meta["perm"], meta["dinv"]

    x_scaled = (np.asarray(x, np.float32) * dinv[:, None]).astype(np.float16)
    table = np.ascontiguousarray(x_scaled[perm])
    dinv_p = dinv[perm]
    jc = np.tile(np.arange(P, dtype=np.float16)[None, :], (P, 1))
    ident = np.eye(P, dtype=np.float16)

    w1h = np.asarray(W1, np.float16)
    w2h = np.asarray(W2, np.float16)
    wfh = np.asarray(Wf, np.float16)
    b1c = np.asarray(b1, np.float32).reshape(-1, 1)
    b2c = np.asarray(b2, np.float32).reshape(-1, 1)
    bfc = np.asarray(bf, np.float32).reshape(-1, 1)

    in_maps = []
    for c in range(n_cores):
        sl = slice(c * shard, (c + 1) * shard)
        in_maps.append(
            {
                "x_shard": np.ascontiguousarray(table[sl]),
                "idx16": np.ascontiguousarray(meta["idx_w"][c]),
                "dst_loc": np.ascontiguousarray(meta["dst_loc"][c]),
                "dinv16": np.ascontiguousarray(
                    np.tile(dinv_p[sl].astype(np.float16)[None, :], (16, 1))
                ),
                "j_const": jc,
                "ident_in": ident,
                "w1": w1h, "w2": w2h, "wf": wfh,
                "b1": b1c, "b2": b2c, "bf": bfc,
            }
        )
    return in_maps


# ----------------------------------------------------------------- timing
def _timed_run(nc, in_maps, n_cores, iters=5):
    """Replicates bass2jax.run_bass_via_pjrt's multi-core path but keeps the
    inputs device-resident so repeated executions approximate pure HW time.
    Returns (per-core results list, list of per-call seconds)."""
    import time

    import jax
    import jax.core
    from jax.experimental.shard_map import shard_map
    from jax.sharding import Mesh, NamedSharding, PartitionSpec

    from concourse import bass2jax, mybir

    bass2jax.install_neuronx_cc_hook()

    partition_name = (
        nc.partition_id_tensor.name if nc.partition_id_tensor else None
    )
    in_names, out_names, out_avals, zero_outs = [], [], [], []
    for alloc in nc.m.functions[0].allocations:
        if not isinstance(alloc, mybir.MemoryLocationSet):
            continue
        name = alloc.memorylocations[0].name
        if alloc.kind == "ExternalInput":
            if name != partition_name:
                in_names.append(name)
        elif alloc.kind == "ExternalOutput":
            shape = tuple(alloc.tensor_shape)
            dtype = mybir.dt.np(alloc.dtype)
            out_names.append(name)
            out_avals.append(jax.core.ShapedArray(shape, dtype))
            zero_outs.append(np.zeros(shape, dtype))
    n_params = len(in_names)
    n_outs = len(out_avals)
    in_names = in_names + out_names
    if partition_name is not None:
        in_names.append(partition_name)
    donate = tuple(range(n_params, n_params + n_outs))

    def _body(*args):
        operands = list(args)
        if partition_name is not None:
            operands.append(bass2jax.partition_id_tensor())
        outs = bass2jax._bass_exec_p.bind(
            *operands,
            out_avals=tuple(out_avals),
            in_names=tuple(in_names),
            out_names=tuple(out_names),
            lowering_input_output_aliases=(),
            sim_require_finite=True,
            sim_require_nnan=True,
            nc=nc,
        )
        return tuple(outs)

    devices = jax.devices()[:n_cores]
    mesh = Mesh(np.asarray(devices), ("core",))
    sharding = NamedSharding(mesh, PartitionSpec("core"))
    sharded = jax.jit(
        shard_map(
            _body,
            mesh=mesh,
            in_specs=(PartitionSpec("core"),) * (n_params + n_outs),
            out_specs=(PartitionSpec("core"),) * len(out_names),
            check_rep=False,
        ),
        donate_argnums=donate,
        keep_unused=True,
    )
    concat_in = [
        np.concatenate(
            [np.asarray(in_maps[c][nm]) for c in range(n_cores)], axis=0
        )
        for nm in in_names[:n_params]
    ]
    dev_in = [jax.device_put(a, sharding) for a in concat_in]
    big_zeros = [
        np.zeros((n_cores * z.shape[0], *z.shape[1:]), z.dtype)
        for z in zero_outs
    ]

    def zeros_on_dev():
        return [jax.device_put(z, sharding) for z in big_zeros]

    out_arrs = sharded(*dev_in, *zeros_on_dev())
    jax.block_until_ready(out_arrs)
    results = [
        {
            nm: np.asarray(out_arrs[i]).reshape(n_cores, *out_avals[i].shape)[c]
            for i, nm in enumerate(out_names)
        }
        for c in range(n_cores)
    ]

    times = []
    pre = [zeros_on_dev() for _ in range(iters)]
    jax.block_until_ready(pre)
    for it in range(iters):
        t0 = time.perf_counter()
        o = sharded(*dev_in, *pre[it])
        jax.block_until_ready(o)
        times.append(time.perf_counter() - t0)
    return results, times


# ------------------------------------------------------------------- entry
def kernel(x, edge_index, W1, b1, W2, b2, Wf, bf):
    from concourse import bass_utils

    x = np.asarray(x)
    edge_index = np.asarray(edge_index)
    meta = _preprocess(x, edge_index)
    fout = np.asarray(Wf).shape[1]

    nc = _build(meta, fout)
    in_maps = _make_in_maps(meta, x, W1, b1, W2, b2, Wf, bf)

    iters = int(os.environ.get("GCN_BENCH_ITERS", "0"))
    if iters > 0:
        results, times = _timed_run(nc, in_maps, meta["n_cores"], iters=iters)
        _LAST["times"] = times
        _LAST["exec_time_ns"] = int(min(times) * 1e9)
    else:
        res = bass_utils.run_bass_kernel_spmd(
            nc,
            in_maps,
            core_ids=list(range(meta["n_cores"])),
            trace=False,
        )
        results = res.results
        _LAST["exec_time_ns"] = res.exec_time_ns
    _LAST["meta"] = meta

    N, shard = meta["N"], meta["shard"]
    out = np.empty((N, fout), dtype=np.float32)
    for c in range(meta["n_cores"]):
        sl = slice(c * shard, (c + 1) * shard)
        out[meta["perm"][sl]] = results[c]["outT"].T
    return out



# revision 47
# speedup vs baseline: 1.0019x; 1.0019x over previous
"""Bass/Trainium2 kernel for a 3-layer GCN (GCNConv x2 + Linear).

Contract: kernel(**inputs) takes the FULL unsharded inputs
(x [N,128] f32, edge_index [2,E] i64, W1,b1,W2,b2,Wf,bf) and returns the
FULL [N,64] f32 output, distributing work across 8 NeuronCores internally.

Math: PyG GCNConv with self loops,
    gcn(x) = Dinv (A + I) Dinv (x W) + b,   Dinv = diag(1/sqrt(deg))
Aggregation and the dense transform commute, so each layer is computed as
    z = dinv * (A @ (dinv * h) + dinv * h);  h' = relu(z @ W + b)
The device gathers rows of a pre-scaled fp16 table (dma_gather on 4 SWDGE
queues, int16 bank-local indices), scatter-adds them into PSUM via one-hot
matmuls on the TensorEngine (one wide broadcast-AP is_equal per
(group, bank) builds all one-hots), applies the self term + dinv scale on
DVE, and runs the small dense matmul locally per 512-node group.

Sharding: destination nodes are sharded 8 ways with a degree-stratified
permutation (nodes dealt round-robin by in-degree to cores, then to tiles)
so per-(slot, bank) segment capacities are near-uniform across the 8 SPMD
cores; segments are 16-index granular with idx-0/dst-300 padding.  Each
core uploads only its own x shard; bank stripes of the table are built
on-device by per-bank AllGathers (and likewise for the layer-2 table,
letting stripes fire as layer-1 groups complete).  The final output is
written feature-major and un-permuted on the host.
"""

import os

import numpy as np

P = 128
N_CORES = 8
GW = 512         # dense-matmul group width = 4 dst tiles (one PSUM bank)
# dma_gather idx is int16: bank the table. 20000 -> 5 banks, which keeps
# each (group, bank) index span under 2048 = two full single_packet calls.
BANK_MAX = int(os.environ.get("GCN_BANK_MAX", "20000"))
CAP_CHUNKS = int(os.environ.get("GCN_CAP_CHUNKS", "8"))  # max chunks/gather
# single_packet packs each SDMA engine's descriptors into one packet (fast
# Q7 generation) but is limited to 64 descs/engine = 1024 indices/gather.
SINGLE_PACKET = os.environ.get("GCN_SINGLE_PACKET", "1") == "1"
N_SWDGE_QUEUES = int(os.environ.get("GCN_SWDGE_QUEUES", "4"))
DMA_SCRATCH = int(os.environ.get("GCN_DMA_SCRATCH", "16384"))

_LAST = {}  # diagnostics from the most recent kernel() call


# ----------------------------------------------------------------- host prep
def _preprocess(x, edge_index, n_cores=N_CORES, bank_max=BANK_MAX):
    N, F = x.shape
    assert N % n_cores == 0
    shard = N // n_cores
    n_tiles = (shard + P - 1) // P
    last_nk = shard - (n_tiles - 1) * P
    n_groups = (shard + GW - 1) // GW
    n_banks = max(1, -(-N // bank_max))
    bank_size = -(-N // n_banks)

    qrows = shard // n_banks  # rows per core per bank stripe
    assert qrows * n_banks == shard

    src = np.asarray(edge_index[0], dtype=np.int64)
    dst = np.asarray(edge_index[1], dtype=np.int64)

    deg = np.bincount(dst, minlength=N).astype(np.float32) + 1.0
    dinv = (1.0 / np.sqrt(deg)).astype(np.float32)

    # stratified permutation: nodes sorted by in-degree desc, dealt
    # round-robin to cores (cross-core balance), then within each core dealt
    # round-robin to tiles (per-tile balance) so every (core, tile) has a
    # near-identical degree mix -> uniform segment capacities.
    deg_order = np.argsort(-deg, kind="stable")
    nk_of_slot = np.full(n_tiles, P, dtype=np.int64)
    nk_of_slot[n_tiles - 1] = last_nk
    pairs = [
        (pos, slot)
        for pos in range(P)
        for slot in range(n_tiles)
        if pos < nk_of_slot[slot]
    ]
    assert len(pairs) == shard
    perm = np.zeros(N, dtype=np.int64)
    for j, (pos, slot) in enumerate(pairs):
        for c in range(n_cores):
            perm[c * shard + slot * P + pos] = deg_order[n_cores * j + c]
    perm_inv = np.zeros(N, dtype=np.int64)
    perm_inv[perm] = np.arange(N)

    new_src = perm_inv[src]
    new_dst = perm_inv[dst]
    core_of = new_dst // shard
    slot_of = (new_dst % shard) // P
    dloc_of = (new_dst % shard) % P
    # bank b = stripe {rows c*shard + b*qrows + [0, qrows) for all c}, laid
    # out rank-major [c*qrows + r] — exactly one AllGather's output. Bank
    # tables have n_cores*qrows rows, which must stay int16-addressable.
    assert n_cores * qrows <= 32767
    e_bank = (new_src % shard) // qrows
    e_bidx = (new_src // shard) * qrows + (new_src % shard) % qrows

    # per-(core, slot, bank) segment counts
    seg = np.zeros((n_cores, n_tiles, n_banks), dtype=np.int64)
    for c in range(n_cores):
        m = core_of == c
        sb = slot_of[m] * n_banks + e_bank[m]
        seg[c] = np.bincount(sb, minlength=n_tiles * n_banks).reshape(
            n_tiles, n_banks
        )
    assert seg.sum(axis=(1, 2)).min() > 0

    # capacity per (slot, bank): max over cores, rounded up to 16 indices
    cap16 = ((seg.max(axis=0) + 15) // 16) * 16  # [n_tiles, n_banks]

    # idx-space layout: for g: for b: slots of g packed back-to-back
    off16 = np.zeros((n_tiles, n_banks), dtype=np.int64)  # local in (g,b)
    gb_q0 = np.zeros((n_groups, n_banks), dtype=np.int64)  # global idx start
    gb_span16 = np.zeros((n_groups, n_banks), dtype=np.int64)
    gb_nblk = np.zeros((n_groups, n_banks), dtype=np.int64)
    qtot = 0
    for g in range(n_groups):
        k_lo, k_hi = 4 * g, min(4 * g + 4, n_tiles)
        for b in range(n_banks):
            # pad the span to a 128 multiple (via the last slot's capacity)
            # so every payload block is fully gathered — ungathered SBUF can
            # be NaN, and NaN * 0 would poison the masked matmul.
            span = int(cap16[k_lo:k_hi, b].sum())
            cap16[k_hi - 1, b] += (-span) % P
            gb_q0[g, b] = qtot
            o = 0
            for k in range(k_lo, k_hi):
                off16[k, b] = o
                o += cap16[k, b]
            gb_span16[g, b] = o
            gb_nblk[g, b] = o // P
            qtot += o

    # matmul pairs: (g, b, slot, 128-block) -> S column m; contiguous per
    # (g, b) so one wide is_equal generates all of a (g, b)'s one-hots.
    gb_m0 = np.zeros((n_groups, n_banks), dtype=np.int64)
    gb_mcnt = np.zeros((n_groups, n_banks), dtype=np.int64)
    pair_of = {}
    mm_seq = {}  # (g, k) -> list of (b, payload block, S block)
    M = 0
    for g in range(n_groups):
        k_lo, k_hi = 4 * g, min(4 * g + 4, n_tiles)
        for b in range(n_banks):
            gb_m0[g, b] = M
            for k in range(k_lo, k_hi):
                if cap16[k, b] == 0:
                    continue
                blo = off16[k, b] // P
                bhi = (off16[k, b] + cap16[k, b] - 1) // P
                for ch in range(blo, bhi + 1):
                    pair_of[(g, b, k, ch)] = M
                    M += 1
            gb_mcnt[g, b] = M - gb_m0[g, b]
        for k in range(k_lo, k_hi):
            sq = []
            for b in range(n_banks):
                if cap16[k, b] == 0:
                    continue
                blo = off16[k, b] // P
                bhi = (off16[k, b] + cap16[k, b] - 1) // P
                for ch in range(blo, bhi + 1):
                    sq.append((b, ch, pair_of[(g, b, k, ch)] - gb_m0[g, b]))
            mm_seq[(g, k)] = sq

    # fill idx and dst_loc per core (pad positions: idx 0, dst_loc 300)
    n_qcols = qtot // 16
    idx_w = np.zeros((n_cores, 16, n_qcols), dtype=np.int16)
    dst_loc = np.full((n_cores, P, M), 300.0, dtype=np.float16)
    g_of_slot = np.arange(n_tiles) // 4
    for c in range(n_cores):
        m = np.where(core_of == c)[0]
        o = m[np.lexsort((e_bank[m], slot_of[m]))]
        ks, bs, rows, dl = slot_of[o], e_bank[o], e_bidx[o], dloc_of[o]
        sb = ks * n_banks + bs
        seg_sizes = np.bincount(sb, minlength=n_tiles * n_banks)
        seg_off = np.concatenate([[0], np.cumsum(seg_sizes)])
        j = np.arange(len(o)) - seg_off[sb]  # rank within segment
        gs_ = g_of_slot[ks]
        ql = off16[ks, bs] + j  # local idx position within (g, b)
        q = gb_q0[gs_, bs] + ql  # global idx position
        idx_w[c, q % 16, q // 16] = rows.astype(np.int16)
        mcols = np.array(
            [pair_of[(gs_[i], bs[i], ks[i], ql[i] // P)]
             for i in range(len(o))],
            dtype=np.int64,
        )
        dst_loc[c, ql % P, mcols] = dl

    return dict(
        N=N, F=F, E=len(src), n_cores=n_cores, shard=shard, n_tiles=n_tiles,
        last_nk=last_nk, nk_of_slot=nk_of_slot, n_groups=n_groups,
        n_banks=n_banks, bank_size=bank_size, qrows=qrows,
        cap16=cap16, off16=off16, gb_q0=gb_q0, gb_span16=gb_span16,
        gb_nblk=gb_nblk, gb_m0=gb_m0, gb_mcnt=gb_mcnt, mm_seq=mm_seq,
        n_qcols=n_qcols, M=M, qtot=qtot,
        perm=perm, perm_inv=perm_inv, dst_loc=dst_loc, idx_w=idx_w,
        dinv=dinv, pad_overhead=qtot * n_cores / max(len(src), 1),
    )


# ------------------------------------------------------------ device program
def _build(meta, fout, debug=False, enable_asserts=False, dbg_outs=False):
    from concourse import bacc, bass, mybir, tile

    dt = mybir.dt
    f16, f32, i16 = dt.float16, dt.float32, dt.int16
    Alu = mybir.AluOpType
    Act = mybir.ActivationFunctionType

    N, F = meta["N"], meta["F"]
    shard, n_tiles = meta["shard"], meta["n_tiles"]
    nk_of_slot = meta["nk_of_slot"]
    n_groups, n_banks = meta["n_groups"], meta["n_banks"]
    bank_size = meta["bank_size"]
    gb_q0, gb_span16 = meta["gb_q0"], meta["gb_span16"]
    gb_nblk = meta["gb_nblk"]
    gb_m0, gb_mcnt = meta["gb_m0"], meta["gb_mcnt"]
    mm_seq = meta["mm_seq"]
    n_qcols, M = meta["n_qcols"], meta["M"]
    n_cores = meta["n_cores"]
    max_nblk = int(gb_nblk.max())
    max_mcnt = int(gb_mcnt.max())

    nc = bacc.Bacc(
        "TRN2",
        target_bir_lowering=False,
        debug=debug,
        enable_asserts=enable_asserts,
        num_devices=n_cores,
        num_swdge_queues=N_SWDGE_QUEUES,
        dynamic_dma_scratch_size=DMA_SCRATCH,
    )

    x_shard_in = nc.dram_tensor("x_shard", [shard, F], f16, kind="ExternalInput")
    idx16 = nc.dram_tensor("idx16", [16, n_qcols], i16, kind="ExternalInput")
    dst_loc = nc.dram_tensor("dst_loc", [P, M], f16, kind="ExternalInput")
    dinv16 = nc.dram_tensor("dinv16", [16, shard], f16, kind="ExternalInput")
    j_const = nc.dram_tensor("j_const", [P, P], f16, kind="ExternalInput")
    ident_in = nc.dram_tensor("ident_in", [P, P], f16, kind="ExternalInput")
    w1 = nc.dram_tensor("w1", [F, F], f16, kind="ExternalInput")
    w2 = nc.dram_tensor("w2", [F, F], f16, kind="ExternalInput")
    wf = nc.dram_tensor("wf", [F, fout], f16, kind="ExternalInput")
    b1 = nc.dram_tensor("b1", [F, 1], f32, kind="ExternalInput")
    b2 = nc.dram_tensor("b2", [F, 1], f32, kind="ExternalInput")
    bf = nc.dram_tensor("bf", [fout, 1], f32, kind="ExternalInput")
    outT = nc.dram_tensor("outT", [fout, shard], f32, kind="ExternalOutput")

    qrows = meta["qrows"]
    bank_rows = n_cores * qrows
    shard_dram = nc.dram_tensor("shard_dram", [shard, F], f16)
    x_stage = nc.dram_tensor("x_stage", [shard, F], f16)
    x_q = [
        nc.dram_tensor(f"x_q{q}", [bank_rows, F], f16, addr_space="Shared")
        for q in range(n_banks)
    ]
    s2_q = [
        nc.dram_tensor(f"s2_q{q}", [bank_rows, F], f16, addr_space="Shared")
        for q in range(n_banks)
    ]

    def bank_ap(table, b):
        return table[b][:, :]

    with tile.TileContext(nc) as tc:
        with (
            tc.tile_pool(name="res", bufs=1) as res,
            tc.tile_pool(name="gat", bufs=2 * n_banks) as gat,
            tc.tile_pool(name="sgen", bufs=n_banks + 2) as sgen,
            tc.tile_pool(name="stg", bufs=4) as stg,
            tc.tile_pool(name="zp", bufs=3) as zp,
            tc.tile_pool(name="h2p", bufs=3) as h2p,
            tc.tile_pool(name="xgp", bufs=8) as xgp,
            tc.tile_pool(name="ps_agg", bufs=3, space="PSUM") as ps_agg,
            tc.tile_pool(name="ps_mm", bufs=2, space="PSUM") as ps_mm,
            tc.tile_pool(name="ps_tp", bufs=2, space="PSUM") as ps_tp,
        ):
            # ---- residents
            dl_sb = res.tile([P, M], f16, name="dl_sb")
            j_sb = res.tile([P, P], f16, name="j_sb")
            ident = res.tile([P, P], f16, name="ident")
            dinv_sb = res.tile([P, shard], f16, name="dinv_sb")
            sT1 = res.tile([P, shard], f16, name="sT1")
            sT2 = res.tile([P, shard], f16, name="sT2")
            ix_res = res.tile([P, n_qcols], i16, name="ix_res")
            w1_sb = res.tile([F, F], f16, name="w1_sb")
            w2_sb = res.tile([F, F], f16, name="w2_sb")
            wf_sb = res.tile([F, fout], f16, name="wf_sb")
            b1_sb = res.tile([F, 1], f32, name="b1_sb")
            b2_sb = res.tile([F, 1], f32, name="b2_sb")
            bf_sb = res.tile([fout, 1], f32, name="bf_sb")
            # replicate the table from per-core shards, one AllGather per
            # bank stripe, first thing (collectives cannot read IO tensors
            # directly -> stage through internal dram). Everything below
            # overlaps with the collectives.
            nc.sync.dma_start(out=x_stage[:, :], in_=x_shard_in[:, :])
            for q in range(n_banks):
                nc.gpsimd.collective_compute(
                    "AllGather",
                    mybir.AluOpType.bypass,
                    replica_groups=[list(range(n_cores))],
                    ins=[x_stage[q * qrows : (q + 1) * qrows, :]],
                    outs=[x_q[q].ap().opt()],
                )
            # indices + dinv: replicate the 16-row band to all 128 partitions
            for r in range(8):
                nc.sync.dma_start(
                    out=ix_res[16 * r : 16 * (r + 1), :], in_=idx16[:, :]
                )
                nc.sync.dma_start(
                    out=dinv_sb[16 * r : 16 * (r + 1), :], in_=dinv16[:, :]
                )
            for sb, dr in [
                (dl_sb, dst_loc), (j_sb, j_const), (ident, ident_in),
                (w1_sb, w1), (w2_sb, w2), (wf_sb, wf),
                (b1_sb, b1), (b2_sb, b2), (bf_sb, bf),
            ]:
                nc.sync.dma_start(out=sb[:], in_=dr[:, :])

            # self-term for layer 1: transpose the local shard on-chip.
            # Loads go through the scalar queue to keep Sync free.
            for k in range(n_tiles):
                nk = int(nk_of_slot[k])
                lo = k * P
                xt = xgp.tile([P, P], f16, name="xt", tag="xg")
                nc.scalar.dma_start(out=xt[:nk, :],
                                    in_=x_shard_in[lo : lo + nk, :])
                tp0 = ps_tp.tile([P, P], f16, name="tp0", tag="tp")
                nc.tensor.transpose(
                    out=tp0[:, :nk], in_=xt[:nk, :], identity=ident[:nk, :nk]
                )
                nc.scalar.activation(
                    out=sT1[:, lo : lo + nk], in_=tp0[:, :nk], func=Act.Copy
                )

            dbg = os.environ.get("GCN_DBG_MODE", "")
            qctr = [0]

            def emit_gather(g, b, table):
                span16 = int(gb_span16[g, b])
                if span16 == 0:
                    return None
                q0 = int(gb_q0[g, b])
                gt = gat.tile([P, max_nblk * F], f16, name="gt", tag="gt")
                s = 0
                while s < span16:
                    n = min(CAP_CHUNKS * P, span16 - s)
                    nblk = -(-n // P)
                    nc.gpsimd.dma_gather(
                        gt[:, (s // P) * F : (s // P + nblk) * F]
                        .rearrange("p (c f) -> p c f", f=F),
                        bank_ap(table, b),
                        ix_res[:, (q0 + s) // 16 : (q0 + s + n) // 16],
                        n, n, F,
                        single_packet=SINGLE_PACKET,
                        queue_num=qctr[0] % N_SWDGE_QUEUES,
                    )
                    qctr[0] += 1
                    s += n
                return gt

            def emit_layer(layer, table, w_sb, b_sb):
                # Head pre-gather: the table's bank AllGathers complete
                # serially, so group 0's bank-major gathers would leave the
                # DMA engines idle between AG completions. Interleaving the
                # first two groups bank-major doubles the work unlocked per
                # AG completion (the gat pool holds exactly two groups).
                n_pre = min(2, n_groups)
                pre = {}
                for b in range(n_banks):
                    for g in range(n_pre):
                        pre[(g, b)] = emit_gather(g, b, table)
                for g in range(n_groups):
                    gs = g * GW
                    ge = min(gs + GW, shard)
                    gw = ge - gs
                    k_lo, k_hi = 4 * g, min(4 * g + 4, n_tiles)
                    # gathers for this group, one tile per bank
                    if g < n_pre:
                        gts = {
                            b: pre[(g, b)]
                            for b in range(n_banks)
                            if pre[(g, b)] is not None
                        }
                    else:
                        gts = {}
                        for b in range(n_banks):
                            gt = emit_gather(g, b, table)
                            if gt is not None:
                                gts[b] = gt
                    if dbg == "gonly":
                        continue
                    # one-hot S tiles for a whole (group, bank) in a single
                    # wide DVE op: S[p, m, j] = (j == dl[p, m]).
                    sgs = {}
                    for b in range(n_banks):
                        mcnt = int(gb_mcnt[g, b])
                        if mcnt == 0:
                            continue
                        m0 = int(gb_m0[g, b])
                        st = sgen.tile([P, max_mcnt * P], f16,
                                       name="st", tag="S")
                        if dbg != "nosgen":
                            out_ap = st[:, : mcnt * P].rearrange(
                                "p (c j) -> p c j", j=P
                            )
                            in0 = j_sb[:, :].unsqueeze(1).broadcast_to(
                                [P, mcnt, P]
                            )
                            in1 = dl_sb[:, m0 : m0 + mcnt].unsqueeze(
                                2
                            ).broadcast_to([P, mcnt, P])
                            nc.vector.tensor_tensor(
                                out=out_ap, in0=in0, in1=in1,
                                op=Alu.is_equal,
                            )
                        sgs[b] = st
                    # self-term source
                    own = sT1 if layer == 1 else sT2
                    own_lo = gs
                    zg = zp.tile([P, GW], f16, name="zg", tag="zg")
                    ps = ps_agg.tile([P, GW], f32, name="ps", tag="agg")
                    for k in range(k_lo, k_hi):
                        nk = int(nk_of_slot[k])
                        lo = k * P
                        kk = lo - gs  # column offset within the group
                        seq = mm_seq[(g, k)]
                        for i, (b, pos, mrel) in enumerate(seq):
                            nc.tensor.matmul(
                                out=ps[:, kk : kk + P],
                                lhsT=gts[b][:, pos * F : (pos + 1) * F],
                                rhs=sgs[b][:, mrel * P : (mrel + 1) * P],
                                start=(i == 0),
                                stop=(i == len(seq) - 1),
                            )
                    # z = (agg + self) * dinv for the whole group at once
                    ztmp = stg.tile([P, GW], f32, name="ztmp", tag="ztmp")
                    nc.vector.tensor_tensor(
                        out=ztmp[:, :gw],
                        in0=ps[:, :gw],
                        in1=own[:, own_lo : own_lo + gw],
                        op=Alu.add,
                    )
                    nc.vector.tensor_tensor(
                        out=zg[:, :gw],
                        in0=ztmp[:, :gw],
                        in1=dinv_sb[:, gs:ge],
                        op=Alu.mult,
                    )
                    # dense transform for the group
                    hp = ps_mm.tile([P, GW], f32, name="hp", tag="mm")
                    nc.tensor.matmul(
                        out=hp[:, :gw], lhsT=w_sb[:], rhs=zg[:, :gw],
                        start=True, stop=True,
                    )
                    if layer == 1:
                        hs = stg.tile([P, GW], f16, name="hs", tag="hs")
                        nc.scalar.activation(
                            out=hs[:, :gw], in_=hp[:, :gw], func=Act.Relu,
                            bias=b_sb[:, :1],
                        )
                        nc.vector.tensor_tensor(
                            out=sT2[:, gs:ge], in0=hs[:, :gw],
                            in1=dinv_sb[:, gs:ge], op=Alu.mult,
                        )
                        for k in range(k_lo, k_hi):
                            nk = int(nk_of_slot[k])
                            lo = k * P
                            tp = ps_tp.tile([P, P], f16, name="tp", tag="tp")
                            nc.tensor.transpose(
                                out=tp[:nk, :],
                                in_=sT2[:, lo : lo + nk],
                                identity=ident[:],
                            )
                            ts = stg.tile([P, P], f16, name="ts", tag="ts")
                            nc.scalar.activation(
                                out=ts[:nk, :], in_=tp[:nk, :],
                                func=Act.Copy,
                            )
                            nc.sync.dma_start(
                                out=shard_dram[lo : lo + nk, :],
                                in_=ts[:nk, :],
                            )
                    else:
                        h2g = h2p.tile([P, GW], f16, name="h2g", tag="h2")
                        nc.scalar.activation(
                            out=h2g[:, :gw], in_=hp[:, :gw], func=Act.Relu,
                            bias=b_sb[:, :1],
                        )
                        op = ps_mm.tile([fout, GW], f32, name="op", tag="mm")
                        nc.tensor.matmul(
                            out=op[:, :gw], lhsT=wf_sb[:], rhs=h2g[:, :gw],
                            start=True, stop=True,
                        )
                        os_ = stg.tile([fout, GW], f32, name="os_", tag="os")
                        nc.scalar.activation(
                            out=os_[:, :gw], in_=op[:, :gw],
                            func=Act.Identity, bias=bf_sb[:, :1],
                        )
                        nc.sync.dma_start(out=outT[:, gs:ge],
                                          in_=os_[:, :gw])

            dbg_mode = os.environ.get("GCN_DBG_MODE", "")
            reps = int(os.environ.get("GCN_REPEAT", "1"))
            for _rep in range(reps):
                emit_layer(1, x_q, w1_sb, b1_sb)
                if dbg_mode != "noag":
                    # per-bank AllGather: each stripe fires as soon as its
                    # shard_dram rows are written, overlapping layer-1
                    # compute; only the last stripe gates layer 2's tail.
                    for q in range(n_banks):
                        nc.gpsimd.collective_compute(
                            "AllGather",
                            mybir.AluOpType.bypass,
                            replica_groups=[list(range(n_cores))],
                            ins=[
                                shard_dram[q * qrows : (q + 1) * qrows, :]
                            ],
                            outs=[s2_q[q].ap().opt()],
                        )
                l2_tab = x_q if dbg_mode in ("noag", "l2x") else s2_q
                emit_layer(2, l2_tab, w2_sb, b2_sb)

            if dbg_outs:
                d_sT2 = nc.dram_tensor("d_sT2", [P, shard], f16,
                                       kind="ExternalOutput")
                nc.sync.dma_start(out=d_sT2[:, :], in_=sT2[:])

    nc.compile()
    return nc


def _make_in_maps(meta, x, W1, b1, W2, b2, Wf, bf):
    shard, n_cores = meta["shard"], meta["n_cores"]
    perm, dinv = meta["perm"], meta["dinv"]

    x_scaled = (np.asarray(x, np.float32) * dinv[:, None]).astype(np.float16)
    table = np.ascontiguousarray(x_scaled[perm])
    dinv_p = dinv[perm]
    jc = np.tile(np.arange(P, dtype=np.float16)[None, :], (P, 1))
    ident = np.eye(P, dtype=np.float16)

    w1h = np.asarray(W1, np.float16)
    w2h = np.asarray(W2, np.float16)
    wfh = np.asarray(Wf, np.float16)
    b1c = np.asarray(b1, np.float32).reshape(-1, 1)
    b2c = np.asarray(b2, np.float32).reshape(-1, 1)
    bfc = np.asarray(bf, np.float32).reshape(-1, 1)

    in_maps = []
    for c in range(n_cores):
        sl = slice(c * shard, (c + 1) * shard)
        in_maps.append(
            {
                "x_shard": np.ascontiguousarray(table[sl]),
                "idx16": np.ascontiguousarray(meta["idx_w"][c]),
                "dst_loc": np.ascontiguousarray(meta["dst_loc"][c]),
                "dinv16": np.ascontiguousarray(
                    np.tile(dinv_p[sl].astype(np.float16)[None, :], (16, 1))
                ),
                "j_const": jc,
                "ident_in": ident,
                "w1": w1h, "w2": w2h, "wf": wfh,
                "b1": b1c, "b2": b2c, "bf": bfc,
            }
        )
    return in_maps


# ----------------------------------------------------------------- timing
def _timed_run(nc, in_maps, n_cores, iters=5):
    """Replicates bass2jax.run_bass_via_pjrt's multi-core path but keeps the
    inputs device-resident so repeated executions approximate pure HW time.
    Returns (per-core results list, list of per-call seconds)."""
    import time

    import jax
    import jax.core
    from jax.experimental.shard_map import shard_map
    from jax.sharding import Mesh, NamedSharding, PartitionSpec

    from concourse import bass2jax, mybir

    bass2jax.install_neuronx_cc_hook()

    partition_name = (
        nc.partition_id_tensor.name if nc.partition_id_tensor else None
    )
    in_names, out_names, out_avals, zero_outs = [], [], [], []
    for alloc in nc.m.functions[0].allocations:
        if not isinstance(alloc, mybir.MemoryLocationSet):
            continue
        name = alloc.memorylocations[0].name
        if alloc.kind == "ExternalInput":
            if name != partition_name:
                in_names.append(name)
        elif alloc.kind == "ExternalOutput":
            shape = tuple(alloc.tensor_shape)
            dtype = mybir.dt.np(alloc.dtype)
            out_names.append(name)
            out_avals.append(jax.core.ShapedArray(shape, dtype))
            zero_outs.append(np.zeros(shape, dtype))
    n_params = len(in_names)
    n_outs = len(out_avals)
    in_names = in_names + out_names
    if partition_name is not None:
        in_names.append(partition_name)
    donate = tuple(range(n_params, n_params + n_outs))

    def _body(*args):
        operands = list(args)
        if partition_name is not None:
            operands.append(bass2jax.partition_id_tensor())
        outs = bass2jax._bass_exec_p.bind(
            *operands,
            out_avals=tuple(out_avals),
            in_names=tuple(in_names),
            out_names=tuple(out_names),
            lowering_input_output_aliases=(),
            sim_require_finite=True,
            sim_require_nnan=True,
            nc=nc,
        )
        return tuple(outs)

    devices = jax.devices()[:n_cores]
    mesh = Mesh(np.asarray(devices), ("core",))
    sharding = NamedSharding(mesh, PartitionSpec("core"))
    sharded = jax.jit(
        shard_map(
            _body,
            mesh=mesh,
            in_specs=(PartitionSpec("core"),) * (n_params + n_outs),
            out_specs=(PartitionSpec("core"),) * len(out_names),
            check_rep=False,
        ),
        donate_argnums=donate,
        keep_unused=True,
    )
    concat_in = [
        np.concatenate(
            [np.asarray(in_maps[c][nm]) for c in range(n_cores)], axis=0
        )
        for nm in in_names[:n_params]
    ]
    dev_in = [jax.device_put(a, sharding) for a in concat_in]
    big_zeros = [
        np.zeros((n_cores * z.shape[0], *z.shape[1:]), z.dtype)
        for z in zero_outs
    ]

    def zeros_on_dev():
        return [jax.device_put(z, sharding) for z in big_zeros]

    out_arrs = sharded(*dev_in, *zeros_on_dev())
    jax.block_until_ready(out_arrs)
    results = [
        {
            nm: np.asarray(out_arrs[i]).reshape(n_cores, *out_avals[i].shape)[c]
            for i, nm in enumerate(out_names)
        }
        for c in range(n_cores)
    ]

    times = []
    pre = [zeros_on_dev() for _ in range(iters)]
    jax.block_until_ready(pre)
    for it in range(iters):
        t0 = time.perf_counter()
        o = sharded(*dev_in, *pre[it])
        jax.block_until_ready(o)
        times.append(time.perf_counter() - t0)
    return results, times


# ------------------------------------------------------------------- entry
def kernel(x, edge_index, W1, b1, W2, b2, Wf, bf):
    from concourse import bass_utils

    x = np.asarray(x)
    edge_index = np.asarray(edge_index)
    meta = _preprocess(x, edge_index)
    fout = np.asarray(Wf).shape[1]

    nc = _build(meta, fout)
    in_maps = _make_in_maps(meta, x, W1, b1, W2, b2, Wf, bf)

    iters = int(os.environ.get("GCN_BENCH_ITERS", "0"))
    if iters > 0:
        results, times = _timed_run(nc, in_maps, meta["n_cores"], iters=iters)
        _LAST["times"] = times
        _LAST["exec_time_ns"] = int(min(times) * 1e9)
    else:
        res = bass_utils.run_bass_kernel_spmd(
            nc,
            in_maps,
            core_ids=list(range(meta["n_cores"])),
            trace=False,
        )
        results = res.results
        _LAST["exec_time_ns"] = res.exec_time_ns
    _LAST["meta"] = meta

    N, shard = meta["N"], meta["shard"]
    out = np.empty((N, fout), dtype=np.float32)
    for c in range(meta["n_cores"]):
        sl = slice(c * shard, (c + 1) * shard)
        out[meta["perm"][sl]] = results[c]["outT"].T
    return out

